# revision 34
# baseline (speedup 1.0000x reference)
"""Trainium2 Bass kernel for a pre-norm transformer decoder layer.

Model: x = x + Attn(LN1(x));  x = x + FFN(LN2(x))
Shapes: x [2, 2048, 1024], H=16 heads, DK=64, FF=4096, f32 I/O.

Sharding over 8 NeuronCores with a cross-core K/V exchange:
  core c -> batch entry b = c//4, query rows q0 = (c%4)*512 .. +512.
  Each core computes K/V only for its OWN 512 tokens, publishes the
  bf16 K^T/V slices to a chip-shared DRAM scratchpad, passes a tiny
  AllGather that acts as a group barrier, then reads the other three
  cores' slices back.  Keys/values land in rotation order
  (block j = group peer (g+j)%4) -- softmax and PV are permutation-
  invariant over keys so no reordering is needed.

Compute: bf16 matmuls with f32 PSUM accumulation; LN stats, softmax
and residuals in f32.  Attention uses S^T-layout scores (k on
partitions) so softmax-exp output E^T feeds the PV matmul directly as
the stationary operand, with a ones-column appended to V to produce
the softmax denominator in the same accumulation.  Q/K biases are
added on the PSUM drain (per-partition bias), the V bias is folded
into bo on the host, and LN rstd uses exp(-0.5*ln(var+eps)) so the
Activation engine stays on one table set (exp/ln/relu/copy).
"""

import numpy as np

import concourse.bass as bass
import concourse.mybir as mybir
import concourse.tile as tile
from concourse.masks import make_identity
from concourse.vector_clock import ScopedClock, VectorClock

F32 = mybir.dt.float32
BF16 = mybir.dt.bfloat16
AF = mybir.ActivationFunctionType
OP = mybir.AluOpType
P = 128


def _bcast(v, p):
    """[D] AP -> [p, D] AP replicated over partitions (step-0 partition dim)."""
    return bass.AP(tensor=v.tensor, offset=v.offset, ap=[[0, p], *v.ap])


class _TC(tile.TileContext):
    """TileContext whose tail drain splits its semaphore waits across
    single-wait NOPs -- this walrus build rejects several sync waits on
    one CTRL instruction ("Too many sync wait commands")."""

    def _drain_and_barrier(self, tick_clock, wait_clock):
        gc = tick_clock.global_clock
        n = len(gc)
        for i in range(n):
            if gc[i] <= 0:
                continue
            sub = [0] * n
            sub[i] = gc[i]
            nop = self.nc.sync.nop(nofuse=True)
            wait_clock.add_sem_waits(nop.ins, ScopedClock({None: VectorClock(sub)}))
        self.nc.sync.drain()
        self.nc.all_engine_barrier()
        popped = self.nc._tile_sem_poison_stack.pop()
        assert popped is self._sem_poison
        self.nc.clear_and_free_semaphores(list(self.sems.allocated().values()))
        self.nc.all_engine_barrier()


def build_program(S=2048, D=1024, H=16, DK=64, FF=4096, Q=512, EPS=1e-6, repeat=1, phases="ABCD", timing_mode=False):
    nd = D // P        # contraction chunks over D
    ns = S // P        # token tiles (full sequence)
    nq = Q // P        # token tiles (query slice)
    nf = FF // P       # chunks over FF
    DK1 = DK + 1       # head dim + denominator column
    HPG = P // DK      # heads per 128-partition group (2)
    n_dh = max(D // 512, 1)   # 512-wide column blocks over D
    DHW = D // n_dh
    NB = S // Q        # key blocks (= group size 4)
    assert H * DK == D and Q <= 512 and NB * Q == S

    nc = bass.Bass("TRN2", num_devices=8)

    if timing_mode:
        # inputs as internal DRAM (no host transfer) -- timing runs only
        def declare_in(name, shape, dtype):
            return nc.dram_tensor(name, shape, dtype)
    else:
        def declare_in(name, shape, dtype):
            return nc.declare_dram_parameter(name, shape, dtype, isOutput=False)

    xq = declare_in("xq", [Q, D], F32)
    if not timing_mode:
        # per-core exchange slot offsets (elements), precomputed on host:
        # [w_kt, w_v, r_kt x3, r_v x3] -- avoids on-device id arithmetic,
        # which burns scarce SP registers
        xoffs = nc.declare_dram_parameter("xoffs", [8, 1], mybir.dt.uint32, isOutput=False)
    # weights arrive pre-folded (LN affines absorbed, bv absorbed into bo)
    # and pre-cast to bf16; bq/bk stay f32 for per-partition drain bias
    Wq = declare_in("Wq", [D, D], BF16)
    bq = declare_in("bq", [D], F32)
    Wk = declare_in("Wk", [D, D], BF16)
    bk = declare_in("bk", [D], F32)
    Wv = declare_in("Wv", [D, D], BF16)
    Wo = declare_in("Wo", [D, D], BF16)
    bo = declare_in("bo", [D], BF16)
    W1 = declare_in("W1", [D, FF], BF16)
    b1 = declare_in("b1", [FF], F32)
    W2 = declare_in("W2", [FF, D], BF16)
    b2 = declare_in("b2", [D], BF16)
    out = nc.declare_dram_parameter("out", [Q, D], F32, isOutput=True)

    # K/V exchange: each core writes its slice to private DRAM, then a
    # shared-output AllGather transports all 8 slices into a pair-shared
    # scratchpad copy (plain DMA stores to "Shared" DRAM are only visible
    # within a NeuronCore pair -- the collective firmware does the real
    # inter-core transport and doubles as the barrier).
    KTSLOT = nd * P * Q          # elems per core's K^T slice
    VROW = H * DK1               # V exchange row: all heads + ones column
    VSLOT = (Q // P) * P * VROW  # elems per core's V slice (ones col included)
    nbuf = 2 if repeat > 1 else 1
    shkt = [nc.dram_tensor(f"shkt{i}", [8 * KTSLOT], BF16, addr_space="Shared")
            for i in range(nbuf)]
    shv = [nc.dram_tensor(f"shv{i}", [8 * VSLOT], BF16, addr_space="Shared")
           for i in range(nbuf)]

    with _TC(nc) as tc:
      with tc.tile_pool(name="const", bufs=1) as cst, \
           tc.tile_pool(name="bar", bufs=2, space="DRAM") as barp:
        ident = cst.tile([P, P], BF16, tag="ident")
        make_identity(nc, ident)
        eps_t = cst.tile([P, 1], F32, tag="eps")
        nc.vector.memset(eps_t, EPS)
        ones_row = cst.tile([1, 512], BF16, tag="ones")
        nc.vector.memset(ones_row, 1.0)
        ones_f = cst.tile([1, P], F32, tag="onesf")
        nc.vector.memset(ones_f, 1.0)
        # per-partition layout of the FFN bias: ff = c*P + p -> [p, c]
        b1_pm = cst.tile([P, nf], F32, tag="b1pm")
        nc.sync.dma_start(out=b1_pm, in_=b1[:].rearrange("(c p) -> p c", p=P))

        # per-partition columns for Q/K drain bias: d = c*P + p -> [p, c]
        bq_pm = cst.tile([P, nd], F32, tag="bqpm")
        nc.sync.dma_start(out=bq_pm, in_=bq[:].rearrange("(c p) -> p c", p=P))
        bk_pm = cst.tile([P, nd], F32, tag="bkpm")
        nc.sync.dma_start(out=bk_pm, in_=bk[:].rearrange("(c p) -> p c", p=P))

        # bias rows (bf16) for PE rank-1 outer-product folding
        bo_row = cst.tile([1, D], BF16, tag="bor")
        nc.sync.dma_start(out=bo_row, in_=_bcast(bo[:], 1))
        b2_row = cst.tile([1, D], BF16, tag="b2r")
        nc.sync.dma_start(out=b2_row, in_=_bcast(b2[:], 1))

        # readback slot offsets (elements).  Real mode: loaded from the
        # per-core xoffs input (rotation peers).  Timing mode: fixed slots
        # (no dynamic DMAs -- walrus registers are scarce, ~6 per engine).
        if timing_mode:
            off_r_kt = [j * KTSLOT for j in range(1, NB)]
            off_r_v = [j * VSLOT for j in range(1, NB)]
        else:
            # one engine per DMA family: each engine brings its own scalar
            # register file and bounds-check register pair
            def load_off(eng, i, name):
                reg = eng.alloc_register(f"xoff_{name}")
                eng.reg_load(reg, xoffs[i:i + 1, 0:1])
                return reg
            off_r_kt = [load_off(nc.scalar, 2 + j, f"rkt{j}") for j in range(NB - 1)]
            off_r_v = [load_off(nc.gpsimd, 5 + j, f"rv{j}") for j in range(NB - 1)]

        for _rep in range(repeat):
            bufi = _rep % nbuf
            kt_d, v_d = shkt[bufi], shv[bufi]
            with tc.tile_pool(name="bc", bufs=1) as bcp:      # O_T: phases B-C
              with tc.tile_pool(name="ab", bufs=1) as abp:    # KT/QT/Vt: phases A-B
                KT = abp.tile([P, nd, S], BF16, tag="kta", name="KT")
                QT = [abp.tile([P, Q], BF16, tag=f"qt{i}", name=f"QT{i}") for i in range(nd)]
                Vt = abp.tile([P, ns, H, DK1], BF16, tag="vta", name="Vt")
                nc.vector.memset(Vt[:, :, :, DK:DK1], 1.0)

                # ---------------- Phase A: LN1 + QKV + K/V exchange ----------------
                if "A" in phases:
                    with (
                        tc.tile_pool(name="xio", bufs=3) as xio,
                        tc.tile_pool(name="xn", bufs=3) as xnp,
                        tc.tile_pool(name="xt", bufs=1) as xtp,
                        tc.tile_pool(name="wbf", bufs=12) as wbfp,
                        tc.tile_pool(name="psA", bufs=3, space="PSUM") as psA,
                        tc.tile_pool(name="psT", bufs=2, space="PSUM") as psT,
                        tc.tile_pool(name="stat", bufs=8) as stp,
                    ):
                        xnT = [xtp.tile([P, Q], BF16, tag=f"xnt{i}", name=f"xnT{i}") for i in range(nd)]

                        def layernorm_tile(x_t, out_bf):
                            """token-major LN (stats+normalize, no affine), bf16 out."""
                            n_sub = max(D // 512, 1)
                            xr = x_t.rearrange("p (n f) -> p n f", n=n_sub)
                            stats = stp.tile([P, n_sub, 6], F32, tag="st", bufs=8, name="stats")
                            for su in range(n_sub):
                                nc.vector.bn_stats(out=stats[:, su, :], in_=xr[:, su, :])
                            mv = stp.tile([P, 2], F32, tag="mv", bufs=8, name="mv")
                            nc.vector.bn_aggr(out=mv, in_=stats)
                            # rstd = exp(-0.5*ln(var+eps)): keeps Act on the
                            # natural_log_exp table set (no reloads for Sqrt)
                            lnv = stp.tile([P, 1], F32, tag="sd", bufs=8, name="lnv")
                            nc.scalar.activation(out=lnv, in_=mv[:, 1:2], func=AF.Ln, bias=eps_t)
                            rstd = stp.tile([P, 1], F32, tag="rs", bufs=8, name="rstd")
                            nc.scalar.activation(out=rstd, in_=lnv, func=AF.Exp, scale=-0.5)
                            nc.vector.tensor_scalar(
                                out=out_bf, in0=x_t, scalar1=mv[:, 0:1], scalar2=rstd,
                                op0=OP.subtract, op1=OP.mult,
                            )

                        # LN1 over own tokens -> xnT (feature-major)
                        for qt in range(nq):
                            x_t = xio.tile([P, D], F32, tag="xt", name="x_t")
                            nc.sync.dma_start(out=x_t, in_=xq[qt * P:(qt + 1) * P, :])
                            xn1 = xnp.tile([P, D], BF16, tag="xn1", name="xn1")
                            layernorm_tile(x_t, xn1)
                            for k in range(nd):
                                pt = psT.tile([P, P], BF16, tag="pt", bufs=2, name="pt")
                                nc.tensor.transpose(pt, xn1[:, k * P:(k + 1) * P], ident)
                                dst = xnT[k][:, qt * P:(qt + 1) * P]
                                if k % 4 == 3:
                                    nc.scalar.activation(out=dst, in_=pt, func=AF.Copy)
                                else:
                                    nc.vector.tensor_copy(dst, pt)

                        # weights arrive pre-folded bf16: direct DMA
                        def load_w(W_h, name):
                            wtiles = []
                            for k in range(nd):
                                wb = wbfp.tile([P, D], BF16, tag="wbf", name=f"wbf_{name}{k}")
                                nc.sync.dma_start(out=wb, in_=W_h[k * P:(k + 1) * P, :])
                                wtiles.append(wb)
                            return wtiles

                        # ---- K^T own = Wk'^T @ xn^T (bias on drain) ----
                        Wk_bf = load_w(Wk, "k")
                        for cg in range(nd):
                            ps = psA.tile([P, Q], F32, tag="ps", bufs=3, name="ps_k")
                            for k in range(nd):
                                nc.tensor.matmul(
                                    ps, Wk_bf[k][:, cg * P:(cg + 1) * P], xnT[k],
                                    start=(k == 0), stop=(k == nd - 1),
                                )
                            dst = KT[:, cg, 0:Q]
                            if cg % 2:
                                nc.scalar.activation(out=dst, in_=ps, func=AF.Identity,
                                                     bias=bk_pm[:, cg:cg + 1])
                            else:
                                nc.vector.tensor_scalar_add(dst, ps, bk_pm[:, cg:cg + 1])
                        kt_local = barp.tile([nd * P * Q], BF16, tag="ktl", name="kt_local")
                        nc.sync.dma_start(
                            out=bass.AP(tensor=kt_local.tensor, offset=kt_local.offset,
                                        ap=[[Q, P], [P * Q, nd], [1, Q]]),
                            in_=KT[:, :, 0:Q])
                        cc_kt = nc.gpsimd.collective_compute(
                            "AllGather", OP.bypass,
                            replica_groups=[[0, 1, 2, 3, 4, 5, 6, 7]],
                            ins=[kt_local[:]], outs=[kt_d[:]],
                        )
                        for i in range(NB - 1):
                            for bb in range(nbuf):
                                tc.chain_iter_dep(f"rb{bb}_{i}", cc_kt.ins)

                        # ---- V own = xn @ Wv' (token-major into Vt[:, 0:nq]) ----
                        Wv_bf = load_w(Wv, "v")
                        for st in range(nq):
                            for hh in range(n_dh):
                                ps = psA.tile([P, DHW], F32, tag="ps", bufs=3, name="ps_v")
                                for k in range(nd):
                                    nc.tensor.matmul(
                                        ps, xnT[k][:, st * P:(st + 1) * P],
                                        Wv_bf[k][:, hh * DHW:(hh + 1) * DHW],
                                        start=(k == 0), stop=(k == nd - 1),
                                    )
                                hpb = DHW // DK  # heads per column block
                                dst = Vt[:, st, hh * hpb:(hh + 1) * hpb, 0:DK]
                                src = ps.rearrange("p (h d) -> p h d", d=DK)
                                if (st * n_dh + hh) % 4 == 3:
                                    nc.scalar.activation(out=dst, in_=src, func=AF.Copy)
                                else:
                                    nc.vector.tensor_copy(dst, src)
                        v_local = barp.tile([nq * P * VROW], BF16, tag="vl", name="v_local")
                        nc.sync.dma_start(
                            out=bass.AP(tensor=v_local.tensor, offset=v_local.offset,
                                        ap=[[VROW, P], [P * VROW, nq], [1, VROW]]),
                            in_=Vt[:, 0:nq, :, :].rearrange("p s h d -> p s (h d)"))
                        cc_v = nc.gpsimd.collective_compute(
                            "AllGather", OP.bypass,
                            replica_groups=[[0, 1, 2, 3, 4, 5, 6, 7]],
                            ins=[v_local[:]], outs=[v_d[:]],
                        )
                        for i in range(NB - 1):
                            for bb in range(nbuf):
                                tc.chain_iter_dep(f"rb{bb}_{NB - 1 + i}", cc_v.ins)

                        # ---- readback: peers' K^T / V slices ----
                        kt_eng = nc.scalar if not timing_mode else nc.sync
                        v_eng = nc.gpsimd if not timing_mode else nc.sync
                        for j in range(1, NB):
                            rkt = kt_eng.dma_start(
                                out=KT[:, :, j * Q:(j + 1) * Q],
                                in_=bass.AP(tensor=kt_d, offset=off_r_kt[j - 1],
                                            ap=[[Q, P], [P * Q, nd], [1, Q]]))
                            tc.chain_iter_dep(f"rb{bufi}_{j - 1}", rkt.ins)
                            rv = v_eng.dma_start(
                                out=Vt[:, j * nq:(j + 1) * nq, :, :].rearrange("p s h d -> p s (h d)"),
                                in_=bass.AP(tensor=v_d, offset=off_r_v[j - 1],
                                            ap=[[VROW, P], [P * VROW, nq], [1, VROW]]))
                            tc.chain_iter_dep(f"rb{bufi}_{NB - 2 + j}", rv.ins)

                        # ---- Q^T = Wq'^T @ xn^T (bias on drain; overlaps barrier) ----
                        Wq_bf = load_w(Wq, "q")
                        for cg in range(nd):
                            ps = psA.tile([P, Q], F32, tag="ps", bufs=3, name="ps_q")
                            for k in range(nd):
                                nc.tensor.matmul(
                                    ps, Wq_bf[k][:, cg * P:(cg + 1) * P], xnT[k],
                                    start=(k == 0), stop=(k == nd - 1),
                                )
                            if cg % 2:
                                nc.scalar.activation(out=QT[cg], in_=ps, func=AF.Identity,
                                                     bias=bq_pm[:, cg:cg + 1])
                            else:
                                nc.vector.tensor_scalar_add(QT[cg], ps, bq_pm[:, cg:cg + 1])

                # ---------------- Phase B: attention ----------------
                O_T = [bcp.tile([P, Q], BF16, tag=f"ot{i}", name=f"O_T{i}") for i in range(nd)]
                if "B" in phases:
                    with (
                        tc.tile_pool(name="psS", bufs=3, space="PSUM") as psS,
                        tc.tile_pool(name="psO", bufs=3, space="PSUM") as psO,
                        tc.tile_pool(name="psT2", bufs=2, space="PSUM") as psT2,
                        tc.tile_pool(name="sc", bufs=8) as scp,
                    ):
                        kpp = 2 if ns % 2 == 0 else 1   # kt tiles per psum/exp group
                        for h in range(H):
                            cg, ro = h // HPG, (h % HPG) * DK
                            # E^T = exp(S^T / sqrt(DK)), S^T = K_h @ Q_h^T
                            e_tiles = []
                            for ktp in range(ns // kpp):
                                ps = psS.tile([P, kpp, Q], F32, tag="pss", bufs=2, name="ps_s")
                                for j in range(kpp):
                                    kt = ktp * kpp + j
                                    nc.tensor.matmul(
                                        ps[:, j, :], KT[ro:ro + DK, cg, kt * P:(kt + 1) * P],
                                        QT[cg][ro:ro + DK, :], start=True, stop=True,
                                    )
                                et = abp.tile([P, kpp, Q], BF16, tag="et", bufs=12, name=f"et{h}_{ktp}")
                                nc.scalar.activation(out=et, in_=ps, func=AF.Exp, scale=float(1.0 / np.sqrt(DK)))
                                e_tiles.append(et)
                            # O^T_h = [V_h | 1]^T @ E^T  (feature-major, N=512 moving);
                            # row DK holds the softmax denominators per query
                            po = psO.tile([DK1, Q], F32, tag="pso", bufs=2, name="ps_o")
                            for kt in range(ns):
                                nc.tensor.matmul(
                                    po, Vt[:, kt, h, :], e_tiles[kt // kpp][:, kt % kpp, :],
                                    start=(kt == 0), stop=(kt == ns - 1),
                                )
                            rrow = scp.tile([1, Q], F32, tag="rr", bufs=4, name="rrow")
                            nc.vector.reciprocal(out=rrow, in_=po[DK:DK1, :])
                            # partition-broadcast 1/denom via PE rank-1 product
                            rbc = psT2.tile([DK, Q], F32, tag="rbc", bufs=2, name="rbc")
                            nc.tensor.matmul(rbc, ones_f[:, :DK], rrow, start=True, stop=True)
                            rbs = scp.tile([DK, Q], F32, tag="rbs", bufs=4, name="rbs")
                            nc.vector.tensor_copy(rbs, rbc)
                            nc.vector.scalar_tensor_tensor(
                                out=O_T[cg][ro:ro + DK, :], in0=po[0:DK, :], scalar=1.0,
                                in1=rbs, op0=OP.mult, op1=OP.mult,
                            )

              # -------------- Phases C+D (x2 / xn2T live in both) --------------
              with tc.tile_pool(name="cd", bufs=1) as ccp:
                x2 = [ccp.tile([P, D], F32, tag=f"x2{i}", name=f"x2_{i}") for i in range(nq)]
                xn2T = [ccp.tile([P, Q], BF16, tag=f"x2t{i}", name=f"xn2T{i}") for i in range(nd)]

                # -------------- Phase C: O-proj + residual + LN2 --------------
                if "C" in phases:
                    with (
                        tc.tile_pool(name="wob", bufs=8) as wob,
                        tc.tile_pool(name="xioc", bufs=2) as xioc,
                        tc.tile_pool(name="psC", bufs=3, space="PSUM") as psC,
                        tc.tile_pool(name="psT3", bufs=2, space="PSUM") as psT3,
                        tc.tile_pool(name="statc", bufs=4) as stc,
                    ):
                        Wo_bf = []
                        for k in range(nd):
                            wb = wob.tile([P, D], BF16, tag="wob", bufs=8, name=f"wo_bf{k}")
                            nc.sync.dma_start(out=wb, in_=Wo[k * P:(k + 1) * P, :])
                            Wo_bf.append(wb)
                        for qt in range(nq):
                            xq_t = xioc.tile([P, D], F32, tag="xqc", name="xq_c")
                            nc.sync.dma_start(out=xq_t, in_=xq[qt * P:(qt + 1) * P, :])
                            for hh in range(n_dh):
                                ps = psC.tile([P, DHW], F32, tag="psc", bufs=3, name="ps_c")
                                nc.tensor.matmul(ps, ones_row[:, :P], bo_row[:, hh * DHW:(hh + 1) * DHW], start=True, stop=False)
                                for k in range(nd):
                                    nc.tensor.matmul(
                                        ps, O_T[k][:, qt * P:(qt + 1) * P],
                                        Wo_bf[k][:, hh * DHW:(hh + 1) * DHW],
                                        start=False, stop=(k == nd - 1),
                                    )
                                nc.vector.tensor_tensor(
                                    out=x2[qt][:, hh * DHW:(hh + 1) * DHW], in0=ps,
                                    in1=xq_t[:, hh * DHW:(hh + 1) * DHW], op=OP.add,
                                )
                            # LN2 (affine folded into W1/b1), then transpose
                            n_sub = max(D // 512, 1)
                            xr = x2[qt].rearrange("p (n f) -> p n f", n=n_sub)
                            stats = stc.tile([P, n_sub, 6], F32, tag="st2", bufs=4, name="stats2")
                            for su in range(n_sub):
                                nc.vector.bn_stats(out=stats[:, su, :], in_=xr[:, su, :])
                            mv = stc.tile([P, 2], F32, tag="mv2", bufs=4, name="mv2")
                            nc.vector.bn_aggr(out=mv, in_=stats)
                            lnv = stc.tile([P, 1], F32, tag="sd2", bufs=4, name="lnv2")
                            nc.scalar.activation(out=lnv, in_=mv[:, 1:2], func=AF.Ln, bias=eps_t)
                            rstd = stc.tile([P, 1], F32, tag="rs2", bufs=4, name="rstd2")
                            nc.scalar.activation(out=rstd, in_=lnv, func=AF.Exp, scale=-0.5)
                            xn2 = stc.tile([P, D], BF16, tag="xn2", bufs=2, name="xn2")
                            nc.vector.tensor_scalar(
                                out=xn2, in0=x2[qt], scalar1=mv[:, 0:1], scalar2=rstd,
                                op0=OP.subtract, op1=OP.mult,
                            )
                            for k in range(nd):
                                pt = psT3.tile([P, P], BF16, tag="pt3", bufs=2, name="pt3")
                                nc.tensor.transpose(pt, xn2[:, k * P:(k + 1) * P], ident)
                                dst = xn2T[k][:, qt * P:(qt + 1) * P]
                                if k % 4 == 3:
                                    nc.scalar.activation(out=dst, in_=pt, func=AF.Copy)
                                else:
                                    nc.vector.tensor_copy(dst, pt)

                # ---------------- Phase D: FFN + residual + out ----------------
                if "D" in phases:
                    with (
                        tc.tile_pool(name="wd", bufs=1) as wd,
                        tc.tile_pool(name="h1", bufs=1) as h1p,
                        tc.tile_pool(name="y2a", bufs=1) as y2p,
                        tc.tile_pool(name="od", bufs=2) as odp,
                        tc.tile_pool(name="psH", bufs=2, space="PSUM") as psH,
                        tc.tile_pool(name="psY", bufs=2, space="PSUM") as psY,
                    ):
                        h1T = [h1p.tile([P, Q], BF16, tag=f"h1{i}", name=f"h1T{i}") for i in range(nf)]
                        y2a = [y2p.tile([P, D], F32, tag=f"ya{i}", name=f"y2a{i}") for i in range(nq)]
                        nfh = max(nf // 2, 1)        # ff chunks per half
                        FFW = nfh * P                # ff columns per half
                        n_w1stage = max(FFW // 1024, 1)
                        W1W = FFW // n_w1stage

                        def load_w1_half(half):
                            tiles = []
                            for k in range(nd):
                                parts = []
                                for j in range(n_w1stage):
                                    c0 = half * FFW + j * W1W
                                    wbt = wd.tile([P, W1W], BF16, tag="w1", bufs=nd * n_w1stage + 6,
                                                  name=f"w1b{half}_{k}_{j}")
                                    nc.sync.dma_start(out=wbt, in_=W1[k * P:(k + 1) * P, c0:c0 + W1W])
                                    parts.append(wbt)
                                tiles.append(parts)
                            return tiles

                        def load_w2_half(half):
                            tiles = []
                            for fc in range(half * nfh, (half + 1) * nfh):
                                wbt = wd.tile([P, D], BF16, tag="w2", bufs=nfh + 2, name=f"w2b{fc}")
                                nc.sync.dma_start(out=wbt, in_=W2[fc * P:(fc + 1) * P, :])
                                tiles.append(wbt)
                            return tiles

                        def h1_half(w1_tiles, half):
                            for fc in range(half * nfh, (half + 1) * nfh):
                                ps = psH.tile([P, Q], F32, tag="psh", bufs=3, name="ps_h")
                                lc = fc - half * nfh
                                j, jo = divmod(lc * P, W1W)
                                for k in range(nd):
                                    nc.tensor.matmul(
                                        ps, w1_tiles[k][j][:, jo:jo + P], xn2T[k],
                                        start=(k == 0), stop=(k == nd - 1),
                                    )
                                if fc % 4 == 3:
                                    nc.scalar.activation(
                                        out=h1T[fc], in_=ps, func=AF.Relu, bias=b1_pm[:, fc:fc + 1],
                                    )
                                else:
                                    # relu(ps + b1) = (ps + b1) max 0 on DVE
                                    nc.vector.tensor_scalar(
                                        out=h1T[fc], in0=ps, scalar1=b1_pm[:, fc:fc + 1],
                                        scalar2=0.0, op0=OP.add, op1=OP.max,
                                    )

                        w1a = load_w1_half(0)
                        h1_half(w1a, 0)
                        w1b = load_w1_half(1)
                        w2a = load_w2_half(0)
                        # y2a = h1[:, :FFW] @ W2[:FFW] (drained to SBUF)
                        for qt in range(nq):
                            for hh in range(n_dh):
                                ps = psY.tile([P, DHW], F32, tag="psy", bufs=2, name="ps_ya")
                                for i, fc in enumerate(range(0, nfh)):
                                    nc.tensor.matmul(
                                        ps, h1T[fc][:, qt * P:(qt + 1) * P],
                                        w2a[i][:, hh * DHW:(hh + 1) * DHW],
                                        start=(i == 0), stop=(i == nfh - 1),
                                    )
                                dst = y2a[qt][:, hh * DHW:(hh + 1) * DHW]
                                if qt % 2:
                                    nc.scalar.activation(out=dst, in_=ps, func=AF.Copy)
                                else:
                                    nc.vector.tensor_copy(dst, ps)
                        h1_half(w1b, 1)
                        w2b = load_w2_half(1)
                        for qt in range(nq):
                            o_t = odp.tile([P, D], F32, tag="od", name="o_t")
                            for hh in range(n_dh):
                                ps = psY.tile([P, DHW], F32, tag="psy", bufs=2, name="ps_yb")
                                nc.tensor.matmul(ps, ones_row[:, :P], b2_row[:, hh * DHW:(hh + 1) * DHW], start=True, stop=False)
                                for i, fc in enumerate(range(nfh, nf)):
                                    nc.tensor.matmul(
                                        ps, h1T[fc][:, qt * P:(qt + 1) * P],
                                        w2b[i][:, hh * DHW:(hh + 1) * DHW],
                                        start=False, stop=(i == nfh - 1),
                                    )
                                sl = slice(hh * DHW, (hh + 1) * DHW)
                                nc.vector.tensor_tensor(out=o_t[:, sl], in0=ps, in1=y2a[qt][:, sl], op=OP.add)
                                nc.vector.tensor_tensor(out=o_t[:, sl], in0=o_t[:, sl], in1=x2[qt][:, sl], op=OP.add)
                            nc.sync.dma_start(out=out[qt * P:(qt + 1) * P, :], in_=o_t)

    return nc


_MAXW = 1  # max sync waits walrus accepts per instruction here


def _split_waits_json(raw: bytes) -> bytes:
    """Split multi-wait instructions: excess sync waits move onto
    preceding single-wait EventSemaphore instructions on the same
    engine (the engine stalls there, gating everything it issues
    afterwards -- semantically identical, codegen-legal)."""
    import json as _json

    d = _json.loads(raw)
    ctr = 0
    for f in d.get("functions", []):
        for bb in f.get("blocks", []):
            insts = bb.get("instructions", [])
            out = []
            for ins in insts:
                si = ins.get("sync_info")
                waits = si.get("on_wait") if si else None
                if waits and len(waits) > _MAXW:
                    for w in waits[:-_MAXW]:
                        ctr += 1
                        out.append({
                            "debug": ins.get("debug", 0),
                            "engine": ins["engine"],
                            "ins": [],
                            "outs": [],
                            "name": f"wsplit-{ctr}",
                            "opcode": "EventSemaphore",
                            "sync_info": {"on_update": [], "on_wait": [w]},
                        })
                    si["on_wait"] = waits[-_MAXW:]
                out.append(ins)
            bb["instructions"] = out
    return _json.dumps(d).encode()


def _patch_serialization(nc):
    orig = nc.to_json_bytes

    def patched():
        return _split_waits_json(orig())

    nc.to_json_bytes = patched
    return nc


_CACHED = {}


def _get_nc():
    if "nc" not in _CACHED:
        _CACHED["nc"] = _patch_serialization(build_program())
    return _CACHED["nc"]


def fold_weights(inputs):
    """Host-side prep: absorb the LN affines into adjacent weights/biases
    (exact f32 algebra), fold the V bias through Wo into bo, then cast
    weights to bf16 for the TensorEngine.

      LN(x) @ W + b = z @ (g*W) + (ln_b @ W + b),  z = (x-mu)*rstd
      softmax(S) @ (V + 1 bv') @ Wo + bo = softmax(S) @ V @ Wo + (bv' Wo + bo)
    """
    import ml_dtypes

    f = lambda k: np.asarray(inputs[k], dtype=np.float32)
    bf = lambda a: np.ascontiguousarray(np.asarray(a, np.float32).astype(ml_dtypes.bfloat16))
    f32c = lambda a: np.ascontiguousarray(np.asarray(a, np.float32))
    g1, l1b = f("ln1_g"), f("ln1_b")
    g2, l2b = f("ln2_g"), f("ln2_b")
    out = {}
    for nm, bnm in (("Wq", "bq"), ("Wk", "bk")):
        W = f(nm)
        out[nm] = bf(g1[:, None] * W)
        out[bnm] = f32c(l1b @ W + f(bnm))
    Wv = f("Wv")
    out["Wv"] = bf(g1[:, None] * Wv)
    bv_full = l1b @ Wv + f("bv")   # V bias, applied post-softmax == pre-Wo
    W1 = f("W1")
    out["W1"] = bf(g2[:, None] * W1)
    out["b1"] = np.ascontiguousarray(l2b @ W1 + f("b1"))
    out["Wo"] = bf(f("Wo"))
    out["bo"] = bf(bv_full @ f("Wo") + f("bo"))
    out["W2"] = bf(f("W2"))
    out["b2"] = bf(f("b2"))
    return out


def make_in_maps(inputs):
    x = np.ascontiguousarray(np.asarray(inputs["x"], dtype=np.float32))
    B, S, D = x.shape
    QW = B * S // 8
    shared = fold_weights(inputs)
    gpb = 8 // B  # cores per batch entry
    D_ = x.shape[2]
    nd = D_ // 128
    KTSLOT = nd * 128 * QW
    H_, DK1_ = 16, 65
    VSLOT = (QW // 128) * 128 * (H_ * DK1_)
    in_maps = []
    for c in range(8):
        b, g = c // gpb, c % gpb
        m = dict(shared)
        m["xq"] = np.ascontiguousarray(x[b][g * QW:(g + 1) * QW])
        base = c - g
        peers = [base + (g + j) % gpb for j in range(1, gpb)]
        offs = ([c * KTSLOT, c * VSLOT]
                + [p * KTSLOT for p in peers] + [p * VSLOT for p in peers])
        m["xoffs"] = np.asarray(offs, np.uint32).reshape(8, 1)
        in_maps.append(m)
    return in_maps


def kernel(**inputs) -> np.ndarray:
    from concourse.bass_utils import run_bass_kernel_spmd

    x = np.asarray(inputs["x"])
    B, S, D = x.shape
    QW = B * S // 8
    gpb = 8 // B
    nc = _get_nc()
    res = run_bass_kernel_spmd(nc, make_in_maps(inputs), core_ids=list(range(8)))
    out = np.empty((B, S, D), dtype=np.float32)
    for c in range(8):
        b, g = c // gpb, c % gpb
        out[b, g * QW:(g + 1) * QW] = res.results[c]["out"]
    return out
